# revision 19
# baseline (speedup 1.0000x reference)
"""Trainium2 Bass kernel for nn_ConvEmbedding.

Computes out = L2normalize_rows(x @ W_band^T + b) where W_band is the
(E, D) banded scatter of the Conv1d weight w (E, K): W_band[i, i+k] = w[i, k].

Strategy (8 NeuronCores, data-parallel over batch N):
  - host: build WbT = W_band.T (D, E) once, packed to only the banded column
    ranges; shard x row-wise into 8 shards of (NSH, D), pre-tiled into the
    exact SBUF layout (contraction dim d on partitions) so each 128-row
    output tile is one large contiguous DMA and no on-chip transpose is
    needed.
  - device (per core): per 128-row output tile, banded matmuls accumulate
    xt_tile^T @ WbT_tile into PSUM, skipping all-zero band tiles. VectorE
    adds the bias (exact fp32, post-matmul), ScalarE does square+row-sum in
    one fused op + sqrt, VectorE does max(., eps), reciprocal and scale;
    DMA writes the tile out in fp16 (upcast on host). Weight/bias preloads
    and out-stores ride the ACT HWDGE ring so they never FIFO-block the xt
    prefetch stream (SP ring). The bias ships as one 2KB row and is
    partition-broadcast on GpSimd; the Sqrt/Square ACT tables are warmed in
    the preamble so hw-loop reps don't reload them (2x1.5us/rep) and the
    one-shot load overlaps the weight DMAs.
"""

import os

import numpy as np

import concourse.mybir as mybir
import concourse.tile as tile
from concourse import bacc
from concourse.bass import ts
from concourse.bass_utils import run_bass_kernel_spmd

N, D, E, KW = 16384, 2048, 512, 1537
EPS = 1e-12
NCORES = 8
NSH = N // NCORES        # 2048 batch rows per core
NT = NSH // 128          # 16 output row tiles per core
KT = D // 128            # 16 contraction tiles

# float32r: single-pass fp32 matmul mode (full PE rate at free dim >= 256).
# float32: exact fp32 (2 half-speed passes -> 4x slower).
# float16/bfloat16: half-size operands, full PE rate.
_DT_BY_NAME = {
    "float32r": mybir.dt.float32r,
    "float32": mybir.dt.float32,
    "bfloat16": mybir.dt.bfloat16,
    "float16": mybir.dt.float16,
}
MM_DT = _DT_BY_NAME[os.environ.get("CONV_EMB_MM_DT", "float16")]
# transport dtype of the output (upcast to fp32 on host after gather;
# fp16 halves the out-store stream — values are L2-normalized, |v| <= 1)
OUT_DT = _DT_BY_NAME[os.environ.get("CONV_EMB_OUT_DT", "float16")]


def _band(kt: int) -> tuple[int, int]:
    """Nonzero e-column range [lo, hi) of WbT rows [128*kt, 128*kt+128)."""
    lo = max(0, 128 * kt - (KW - 1))
    hi = min(E, 128 * kt + 128)
    return lo, hi


def _band_used(kt: int, mm_dt) -> tuple[int, int]:
    """Band range; widened to >=256 cols for fp32r (4x slower below 256)."""
    lo, hi = _band(kt)
    if mm_dt == mybir.dt.float32r and hi - lo < 256:
        if lo == 0:
            hi = min(E, 256)
        else:
            lo = max(0, hi - 256)
    return lo, hi


def _band_layout(mm_dt):
    """Per-kt (lo, hi, packed column offset) and the packed total width."""
    off, out = 0, []
    for kt in range(KT):
        lo, hi = _band_used(kt, mm_dt)
        out.append((lo, hi, off))
        off += hi - lo
    return out, off


# Emission order of contraction tiles: the first must write the full bank
# (start=True clears the whole accumulation group), so a full-width kt leads.
def _kt_order(mm_dt):
    full = [kt for kt in range(KT) if _band_used(kt, mm_dt) == (0, E)]
    first = full[0]
    return [first] + [kt for kt in range(KT) if kt != first]


def build_nc(reps: int = 1, mm_dt=None, hw_loop: bool = False,
             staggered: bool = True, xbufs: int = 16, pbufs: int = 8,
             rbufs: int = 6, out_dt=None, use_ttr: bool = False):
    """Build the per-core Bass program (same SPMD program for all cores).

    reps > 1 repeats the whole compute for timing runs; with hw_loop=True the
    repetition is a For_i hardware loop (small program, any rep count).
    """
    if mm_dt is None:
        mm_dt = MM_DT
    if out_dt is None:
        out_dt = OUT_DT
    layout, totw = _band_layout(mm_dt)

    nc = bacc.Bacc(None, target_bir_lowering=False)
    # xt: pre-tiled x shard, xt[i, p, kt*128 + n] = x_shard[i*128 + n, kt*128 + p]
    xt = nc.dram_tensor("xt", [NT, 128, D], mm_dt, kind="ExternalInput")
    # wbt: band-packed WbT, wbt[p, off_kt + j] = WbT[kt*128 + p, lo_kt + j]
    wbt = nc.dram_tensor("wbt", [128, totw], mm_dt, kind="ExternalInput")
    # bias as a single row, fp32; broadcast across partitions on-chip
    bias = nc.dram_tensor("bias", [1, E], mybir.dt.float32,
                          kind="ExternalInput")
    out = nc.dram_tensor("out", [NSH, E], out_dt, kind="ExternalOutput")

    with tile.TileContext(nc) as tc:
        with (
            tc.tile_pool(name="const", bufs=1) as cpool,
            tc.tile_pool(name="xin", bufs=xbufs) as xpool,
            tc.tile_pool(name="res", bufs=rbufs) as rpool,
            tc.tile_pool(name="psum", bufs=pbufs, space="PSUM") as ppool,
        ):
            # weight/bias preloads go on the ACT HWDGE ring so they stream
            # concurrently with the xt loads on the SP ring; per-kt tiles
            # (-> per-kt deps) loaded in matmul consumption order so the PE
            # ramp only waits for the first tile, bias last (needed latest)
            # single SBUF slab for all wbt tiles, loaded with 2 descriptors:
            # kts 0-3 (the first four in consumption order, dram-contiguous)
            # then the rest — fewer serial descriptor issues on the ACT
            # queue in the program head; range-based deps let the first
            # matmuls start as soon as the first chunk lands
            wslab = cpool.tile([128, totw], mm_dt, tag="wslab")
            cut = layout[4][2]
            nc.scalar.dma_start(wslab[:, 0:cut], wbt[:, 0:cut])
            nc.scalar.dma_start(wslab[:, cut:totw], wbt[:, cut:totw])
            wbt_sbs = [
                wslab[:, off:off + (hi - lo)] for (lo, hi, off) in layout
            ]
            # bias: ship one 2KB row, replicate across partitions on GpSimd
            brow = cpool.tile([1, E], mybir.dt.float32, tag="brow")
            nc.scalar.dma_start(brow[:], bias[:])
            bias_sb = cpool.tile([128, E], mybir.dt.float32)
            nc.gpsimd.partition_broadcast(bias_sb[:], brow[:])
            # warm the ACT table (Sqrt) in the preamble so the For_i body
            # doesn't re-issue a 1.5us ACT_TABLE_LOAD per rep and the
            # one-shot load overlaps the weight DMAs
            wrm = cpool.tile([1, 8], mybir.dt.float32, tag="wrm")
            wacc = cpool.tile([1, 1], mybir.dt.float32, tag="wacc")
            nc.gpsimd.memset(wrm[:], 0.0)
            if not use_ttr:
                nc.scalar.activation(
                    wrm[:], wrm[:], mybir.ActivationFunctionType.Square,
                    accum_out=wacc[:],
                )
            nc.scalar.sqrt(wrm[:], wrm[:])

            def body():
                for i in range(NT):
                    xt_sb = xpool.tile([128, D], mm_dt, tag="xt")
                    nc.sync.dma_start(xt_sb[:], xt[i])

                    ps = ppool.tile([128, E], mybir.dt.float32, tag="ps")
                    order = _kt_order(mm_dt)
                    for j, kt in enumerate(order):
                        lo, hi, _ = layout[kt]
                        nc.tensor.matmul(
                            ps[:, lo:hi],
                            xt_sb[:, ts(kt, 128)],
                            wbt_sbs[kt],
                            start=(j == 0), stop=(j == KT - 1),
                            skip_group_check=True,
                        )

                    pre = rpool.tile([128, E], mybir.dt.float32, tag="pre")
                    nc.vector.tensor_add(pre[:], ps[:], bias_sb[:])
                    ss = rpool.tile([128, 1], mybir.dt.float32, tag="ss")
                    if use_ttr:
                        # fused square + row-sum on DVE (keeps ACT free of
                        # the Square pass + accumulator read; sq is unused)
                        sq = rpool.tile([128, E], mybir.dt.float16, tag="sq")
                        nc.vector.tensor_tensor_reduce(
                            sq[:], pre[:], pre[:], 1.0, 0.0,
                            mybir.AluOpType.mult, mybir.AluOpType.add, ss[:],
                        )
                    else:
                        sq = rpool.tile([128, E], mybir.dt.float32, tag="sq")
                        nc.scalar.activation(
                            sq[:], pre[:],
                            mybir.ActivationFunctionType.Square,
                            accum_out=ss[:],
                        )
                    nrm = rpool.tile([128, 1], mybir.dt.float32, tag="nrm")
                    nc.scalar.sqrt(nrm[:], ss[:])
                    nc.vector.tensor_scalar_max(nrm[:], nrm[:], EPS)
                    inv = rpool.tile([128, 1], mybir.dt.float32, tag="inv")
                    nc.vector.reciprocal(inv[:], nrm[:])
                    ob = rpool.tile([128, E], out_dt, tag="ob")
                    nc.vector.tensor_scalar_mul(ob[:], pre[:], inv[:])
                    # out-stores ride the ACT ring: an out DMA waiting on
                    # compute must not FIFO-block the next xt prefetch
                    nc.scalar.dma_start(out[ts(i, 128), :], ob[:])

            if hw_loop and reps > 1:
                # unroll the loop body to amortize the per-iteration
                # all-engine reset barrier (~8us) across several reps
                unroll = next(
                    (u for u in (8, 4, 2) if (reps - 1) % u == 0), 1
                )
                body()
                with tc.For_i(0, (reps - 1) // unroll, 1,
                              staggered_reset=staggered,
                              hint_engines=tuple(mybir.ALL_ENGINES)):
                    for _u in range(unroll):
                        body()
            else:
                for _rep in range(reps):
                    body()
    nc.finalize()
    return nc


def build_wbt(w: np.ndarray) -> np.ndarray:
    """Scatter w (E, KW) into the transposed banded matrix WbT (D, E)."""
    wbt = np.zeros((D, E), np.float32)
    e_idx = np.arange(E)
    rows = (e_idx[:, None] + np.arange(KW)[None, :]).ravel()
    cols = np.repeat(e_idx, KW)
    wbt[rows, cols] = np.ascontiguousarray(w, dtype=np.float32).ravel()
    return wbt


def make_in_maps(x: np.ndarray, w: np.ndarray, b: np.ndarray, mm_dt=None):
    if mm_dt is None:
        mm_dt = MM_DT
    np_dt = mybir.dt.np(mm_dt)
    layout, totw = _band_layout(mm_dt)

    wbt_full = build_wbt(w)
    wpack = np.zeros((128, totw), np_dt)
    for kt in range(KT):
        lo, hi, off = layout[kt]
        wpack[:, off:off + (hi - lo)] = wbt_full[
            kt * 128:(kt + 1) * 128, lo:hi
        ].astype(np_dt)

    bias = np.ascontiguousarray(b, dtype=np.float32).reshape(1, E)

    xr = np.asarray(x, dtype=np.float32).reshape(NCORES, NT, 128, KT, 128)
    in_maps = []
    for c in range(NCORES):
        # [i, n, kt, p] -> [i, p, kt, n], flattened to [i, p, kt*128+n]
        xt_c = np.ascontiguousarray(
            xr[c].transpose(0, 3, 2, 1).astype(np_dt)
        ).reshape(NT, 128, D)
        in_maps.append({"xt": xt_c, "wbt": wpack, "bias": bias})
    return in_maps


def kernel(x: np.ndarray, w: np.ndarray, b: np.ndarray) -> np.ndarray:
    in_maps = make_in_maps(x, w, b)
    nc = build_nc()
    res = run_bass_kernel_spmd(nc, in_maps, core_ids=list(range(NCORES)))
    return np.concatenate(
        [res.results[c]["out"] for c in range(NCORES)], axis=0
    ).astype(np.float32)

